# revision 24
# baseline (speedup 1.0000x reference)
"""Trainium2 Bass kernel for BilinearInteraction.

Reference math (B=2048, F=32 fields, D=64, P=496 field-pairs):
    for pair p=(i,j):  out[b,p,:] = (v_i @ W[p].T) * v_j
    v_i = feature_emb[:, i, :],  v_j = feature_emb[:, j, :]

Sharding: data-parallel over batch, 8 cores x 256 rows each; W replicated.
The fp32 output is 260MB (32.5MB/core) -> the kernel is output-write bound,
so the device writes bf16 (16.25MB/core) and the host upcasts; combined with
bf16 matmul operands the end-to-end relative error is ~3e-3, well inside the
2e-2 gate.

Per-core dataflow (all static, Tile-scheduled):
  - W is pre-transposed, cast to bf16 and packed host-side into
    wpack[128, 16384]: partitions 0:64 hold pairs 0..255 (cols p*64+e =
    W[p,e,d=partition]), partitions 64:128 hold pairs 256..495. Loaded as
    four resident [128,4096] tiles via column-sliced DMAs in demand order
    (padding columns are never loaded).
  - featT[128, 5632] bf16 = per-field transposed features, the stationary
    matmul operand. A matmul requires lhsT/rhs to share a base partition,
    and pairs 0..255 (partitions 0:64) only ever use first-fields 0..9
    while pairs 256..495 (partitions 64:128) use 9..30 - so partitions
    0:64 hold fields 0..9 (col f*256+b) and partitions 64:128 hold fields
    9..30 (col (f-9)*256+b).
  - featN[256, 2048] bf16 = natural-layout features; the elementwise
    multiplier for consecutive pairs of one group is a contiguous slab.
  - The two partition halves (pairs 0..255 on PE row-tile T0, 256..495 on
    T8; K=64 matmuls auto-lower to 64x128 row-tiled mode) are processed
    as two interleaved stage streams so adjacent matmuls target different
    tiles and stream concurrently. Per (batch-half bc, stage pair): pairs
    grouped into "runs" (same first field, one 64-pair W block, <=16
    pairs); each run = 1-2 matmuls [K=64,M=128]x[N<=512] into one PSUM
    tile, then the PSUM x featN Hadamard product via one of two engine
    paths balanced by GPS_FRAC (DVE TT from PSUM ~104 elem/ns; GpSimd
    cannot read PSUM, so its path is ACT copy PSUM->bf16 ~110 then GpSimd
    TT bf16 ~60; ACT/GpSimd are otherwise idle):
       path X (~64%): DVE  tensor_mul(psum_f32, featN_bf16) -> stage bf16
       path Y (~36%): ACT  copy psum -> tmp bf16;
                      GPS  tensor_mul(tmp, featN_bf16)      -> stage bf16
    Each half-stage completes with one HWDGE DMA to its output row-block
    (the output lands directly in natural [b, p*64+e] layout). Early
    output DMAs ride the sync ring while inputs own the scalar ring; once
    the input stream drains, outputs alternate across both HWDGE rings.
    Half B starts at pair 258 (field 10, low-j multipliers) so its first
    Hadamard doesn't wait for the tail of fn0; pairs 256/257 run as a
    tiny deferred stage per batch half.
"""

from itertools import combinations

import numpy as np

N_CORES = 8
B, F, D = 2048, 32, 64
P = 496
B_SH = B // N_CORES            # 256 batch rows per core
HALF = 256                     # pair index where the partition half flips
RUN = 16                       # max pairs per Hadamard op (2 PSUM banks)
GPS_FRAC = 0.36                # share of elements routed via ACT+GpSimd

# output stages per partition-half as (pair_lo, pair_hi); the two halves run
# on PE row-tiles T0 (partitions 0:64) / T8 (64:128) and are interleaved
# run-by-run so both tiles stream concurrently (~2x PE throughput).
# First stages small to prime the pipe.
_BOUNDS_A = [0, 8, 16, 32, 64, 96, 128, 160, 192, 224, 244, 256]
# B starts at 258 (field 10, low j) so its first multiply doesn't wait for
# the tail of fn0; the two i=9 pairs (256,258) run as a mini-stage at the end
_BOUNDS_B = [258, 264, 272, 288, 320, 352, 384, 416, 448, 472, 488, 496]
STAGES_A = list(zip(_BOUNDS_A[:-1], _BOUNDS_A[1:]))
STAGES_B = list(zip(_BOUNDS_B[:-1], _BOUNDS_B[1:]))

PAIRS = list(combinations(range(F), 2))

_NC_CACHE = {}


def _runs(lo, hi):
    """Runs of consecutive same-group pairs (<=RUN) in [lo,hi), not
    crossing 64-pair W-block boundaries."""
    runs = []
    p = lo
    while p < hi:
        i = PAIRS[p][0]
        e = p
        while (e + 1 < hi and PAIRS[e + 1][0] == i and (e + 1 - p) < RUN
               and (e + 1) % 64 != 0):
            e += 1
        runs.append((p, e - p + 1))
        p = e + 1
    return runs


def _build():
    import concourse.tile as tile
    from concourse import bacc, mybir

    F32 = mybir.dt.float32
    BF16 = mybir.dt.bfloat16
    nc = bacc.Bacc("TRN2", target_bir_lowering=False, debug=False,
                   enable_asserts=False, num_devices=N_CORES)

    wpack = nc.dram_tensor("wpack", [128, 4 * 4096], BF16, kind="ExternalInput").ap()
    featT = nc.dram_tensor("featT", [128, 22 * B_SH], BF16, kind="ExternalInput").ap()
    featN = nc.dram_tensor("featN", [B_SH, F * D], BF16, kind="ExternalInput").ap()
    out = nc.dram_tensor("out", [B_SH, P * D], BF16, kind="ExternalOutput").ap()

    with tile.TileContext(nc) as tc:
        with (
            tc.tile_pool(name="win", bufs=1) as win,
            tc.tile_pool(name="feat", bufs=1) as feat,
            tc.tile_pool(name="stage", bufs=8) as stage_pool,
            tc.tile_pool(name="tmp", bufs=8) as tmp_pool,
            tc.tile_pool(name="psum", bufs=4, space="PSUM") as psum_pool,
        ):
            # resident input tiles ------------------------------------------------
            w = [win.tile([128, 4096], BF16, name=f"w{blk}", tag=f"w{blk}")
                 for blk in range(4)]
            ft = feat.tile([128, 22 * B_SH], BF16, name="ft", tag="ft")
            fn = [feat.tile([128, F * D], BF16, name=f"fn{bc}", tag=f"fn{bc}")
                  for bc in range(2)]

            # issue order = joint demand order of the two interleaved pair
            # streams (A: pairs 0..255 / ft top fields 0..9; B: 256..495 /
            # ft bottom fields 9..30), fine slices first so the first
            # matmuls start ~0.4MB into the input stream. All inputs ride
            # the scalar HWDGE ring; early outputs use the sync ring.
            nc.scalar.dma_start(ft[:, 0:512], featT[:, 0:512])
            nc.scalar.dma_start(w[0][:, 0:512], wpack[:, 0:512])
            nc.scalar.dma_start(fn[0][:, 0:1280], featN[0:128, 0:1280])
            nc.scalar.dma_start(w[0][:, 512:4096], wpack[:, 512:4096])
            nc.scalar.dma_start(fn[0][:, 1280:2048], featN[0:128, 1280:2048])
            nc.scalar.dma_start(ft[:, 512:1536], featT[:, 512:1536])
            nc.scalar.dma_start(w[1][:, :], wpack[:, 4096:8192])
            nc.scalar.dma_start(ft[:, 1536:2816], featT[:, 1536:2816])
            nc.scalar.dma_start(w[2][:, :], wpack[:, 8192:12288])
            nc.scalar.dma_start(fn[1][:, :], featN[128:256, :])
            # top half of w3 is fully used (pairs 192..255); bottom half only
            # to col 15360 (pair 495) - skip the padding
            nc.scalar.dma_start(w[3][0:64, :], wpack[0:64, 12288:16384])
            nc.scalar.dma_start(w[3][64:128, 0:3072], wpack[64:128, 12288:15360])
            # ft top half (fields 0..9) ends at col 2560; cols 2816+ exist
            # only in the bottom half (fields 20..30) - skip the padding
            nc.scalar.dma_start(ft[64:128, 2816:22 * B_SH],
                                featT[64:128, 2816:22 * B_SH])

            # compute + output ----------------------------------------------------
            state = {"el_tot": 0, "el_gps": 0, "ring": 0}

            def out_dma(dst, src, bc, si):
                # inputs own the scalar ring early in bc=0; after that
                # alternate output DMAs across both HWDGE rings
                if bc == 0 and si < 5:
                    nc.sync.dma_start(dst, src)
                else:
                    eng = nc.sync if state["ring"] % 2 == 0 else nc.scalar
                    state["ring"] += 1
                    eng.dma_start(dst, src)

            def emit_run(p0, n, st, lo, bc, tag):
                i, j0 = PAIRS[p0]
                h = p0 // HALF
                po = 64 * h
                fcol = (i - 9 * h) * B_SH       # field col in ft's half
                colbase = (p0 - h * HALF) * D
                blk, bcol = colbase // 4096, colbase % 4096
                ps = psum_pool.tile([128, RUN * D], F32, tag="ps", bufs=4)
                for k in range(0, n, 8):
                    nk = min(8, n - k)
                    nc.tensor.matmul(
                        ps[:, k * D:(k + nk) * D],
                        lhsT=ft[po:po + 64,
                                fcol + bc * 128: fcol + bc * 128 + 128],
                        rhs=w[blk][po:po + 64,
                                   bcol + k * D: bcol + (k + nk) * D],
                        start=True, stop=True,
                    )
                st_sl = st[:, (p0 - lo) * D: (p0 - lo + n) * D]
                fn_sl = fn[bc][:, j0 * D: (j0 + n) * D]
                # two Hadamard paths (DVE ~104 elem/ns from PSUM; ACT copy
                # + GpSimd ~60 elem/ns), balanced by GPS_FRAC
                state["el_tot"] += n
                if state["el_gps"] < GPS_FRAC * state["el_tot"]:
                    state["el_gps"] += n
                    tmp = tmp_pool.tile([128, RUN * D], BF16, tag="tmp")
                    nc.scalar.copy(tmp[:, 0:n * D], ps[:, 0:n * D])
                    nc.gpsimd.tensor_mul(st_sl, tmp[:, 0:n * D], fn_sl)
                else:
                    nc.vector.tensor_mul(st_sl, ps[:, 0:n * D], fn_sl)

            for bc in range(2):
                for si in range(len(STAGES_A)):
                    aLo, aHi = STAGES_A[si]
                    bLo, bHi = STAGES_B[si]
                    runsA = _runs(aLo, aHi)
                    runsB = _runs(bLo, bHi)
                    stA = stage_pool.tile([128, (aHi - aLo) * D], BF16,
                                          tag="stA", bufs=6)
                    stB = stage_pool.tile([128, (bHi - bLo) * D], BF16,
                                          tag="stB", bufs=6)
                    # alternate halves run-by-run: adjacent matmuls target
                    # different PE row-tiles and execute concurrently
                    for ri in range(max(len(runsA), len(runsB))):
                        if ri < len(runsA):
                            emit_run(*runsA[ri], stA, aLo, bc, "psA")
                        if ri < len(runsB):
                            emit_run(*runsB[ri], stB, bLo, bc, "psB")
                    out_dma(out[bc * 128: bc * 128 + 128, aLo * D: aHi * D],
                            stA[:, :], bc, si)
                    out_dma(out[bc * 128: bc * 128 + 128, bLo * D: bHi * D],
                            stB[:, :], bc, si)
                # deferred i=9 pairs (256,258) of this batch half
                stX = stage_pool.tile([128, 2 * D], BF16, tag="stB", bufs=6)
                emit_run(256, 2, stX, 256, bc, "psB")
                out_dma(out[bc * 128: bc * 128 + 128, 256 * D: 258 * D],
                        stX[:, :], bc, 99)
    nc.compile()
    return nc


def _pack_inputs(feature_emb, W):
    import ml_dtypes

    BF = ml_dtypes.bfloat16
    feature_emb = np.ascontiguousarray(feature_emb, dtype=np.float32)
    W = np.ascontiguousarray(W, dtype=np.float32)
    Wt = W.transpose(0, 2, 1)                      # [P, d, e]
    wpack = np.zeros((128, 4 * 4096), dtype=BF)
    wpack[0:64, :] = Wt[0:HALF].transpose(1, 0, 2).reshape(64, HALF * D).astype(BF)
    wpack[64:128, 0:(P - HALF) * D] = (
        Wt[HALF:P].transpose(1, 0, 2).reshape(64, (P - HALF) * D).astype(BF))
    in_maps = []
    for c in range(N_CORES):
        shard = feature_emb[c * B_SH:(c + 1) * B_SH]         # [256, 32, 64]
        # [d, f, b] per-field transposed features
        ftT = shard.transpose(2, 1, 0).astype(BF)            # [64, 32, 256]
        featT = np.zeros((128, 22 * B_SH), dtype=BF)
        # partitions 0:64 <- fields 0..9 (first-fields of pairs 0..255)
        featT[0:64, 0:10 * B_SH] = ftT[:, 0:10].reshape(64, 10 * B_SH)
        # partitions 64:128 <- fields 9..30 (first-fields of pairs 256..495)
        featT[64:128, :] = ftT[:, 9:31].reshape(64, 22 * B_SH)
        in_maps.append({
            "wpack": wpack,
            "featT": featT,
            "featN": np.ascontiguousarray(shard.reshape(B_SH, F * D).astype(BF)),
        })
    return in_maps


def kernel(feature_emb, W, _trace=False):
    from concourse.bass_utils import run_bass_kernel_spmd

    if "nc" not in _NC_CACHE:
        _NC_CACHE["nc"] = _build()
    nc = _NC_CACHE["nc"]
    in_maps = _pack_inputs(feature_emb, W)
    res = run_bass_kernel_spmd(nc, in_maps, core_ids=list(range(N_CORES)),
                               trace=_trace)
    full = np.concatenate(
        [res.results[c]["out"].astype(np.float32) for c in range(N_CORES)], axis=0)
    out = full.reshape(B, P, D)
    if _trace:
        return out, res
    return out



# revision 25
# speedup vs baseline: 1.1463x; 1.1463x over previous
"""Trainium2 Bass kernel for BilinearInteraction.

Reference math (B=2048, F=32 fields, D=64, P=496 field-pairs):
    for pair p=(i,j):  out[b,p,:] = (v_i @ W[p].T) * v_j
    v_i = feature_emb[:, i, :],  v_j = feature_emb[:, j, :]

Sharding: data-parallel over batch, 8 cores x 256 rows each; W replicated.
The fp32 output is 260MB (32.5MB/core) -> the kernel is output-write bound,
so the device writes bf16 (16.25MB/core) and the host upcasts; combined with
bf16 matmul operands the end-to-end relative error is ~3e-3, well inside the
2e-2 gate.

Per-core dataflow (all static, Tile-scheduled):
  - W is pre-transposed, cast to bf16 and packed host-side into
    wpack[128, 16384]: partitions 0:64 hold pairs 0..255 (cols p*64+e =
    W[p,e,d=partition]), partitions 64:128 hold pairs 256..495. Loaded as
    four resident [128,4096] tiles via column-sliced DMAs in demand order
    (padding columns are never loaded).
  - featT[128, 5632] bf16 = per-field transposed features, the stationary
    matmul operand. A matmul requires lhsT/rhs to share a base partition,
    and pairs 0..255 (partitions 0:64) only ever use first-fields 0..9
    while pairs 256..495 (partitions 64:128) use 9..30 - so partitions
    0:64 hold fields 0..9 (col f*256+b) and partitions 64:128 hold fields
    9..30 (col (f-9)*256+b).
  - featN[256, 2048] bf16 = natural-layout features; the elementwise
    multiplier for consecutive pairs of one group is a contiguous slab.
  - The two partition halves (pairs 0..255 on PE row-tile T0, 256..495 on
    T8; K=64 matmuls auto-lower to 64x128 row-tiled mode) are processed
    as two interleaved stage streams so adjacent matmuls target different
    tiles and stream concurrently. Per (batch-half bc, stage pair): pairs
    grouped into "runs" (same first field, one 64-pair W block, <=16
    pairs); each run = 1-2 matmuls [K=64,M=128]x[N<=512] into one PSUM
    tile, then the PSUM x featN Hadamard product via one of two engine
    paths balanced by GPS_FRAC (DVE TT from PSUM ~104 elem/ns; GpSimd
    cannot read PSUM, so its path is ACT copy PSUM->bf16 ~110 then GpSimd
    TT bf16 ~60; ACT/GpSimd are otherwise idle):
       path X (~64%): DVE  tensor_mul(psum_f32, featN_bf16) -> stage bf16
       path Y (~36%): ACT  copy psum -> tmp bf16;
                      GPS  tensor_mul(tmp, featN_bf16)      -> stage bf16
    Each half-stage completes with one HWDGE DMA to its output row-block
    (the output lands directly in natural [b, p*64+e] layout). Early
    output DMAs ride the sync ring while inputs own the scalar ring; once
    the input stream drains, outputs alternate across both HWDGE rings.
    Half B starts at pair 258 (field 10, low-j multipliers) so its first
    Hadamard doesn't wait for the tail of fn0; pairs 256/257 run as a
    tiny deferred stage per batch half.
"""

from itertools import combinations

import numpy as np

N_CORES = 8
B, F, D = 2048, 32, 64
P = 496
B_SH = B // N_CORES            # 256 batch rows per core
HALF = 256                     # pair index where the partition half flips
RUN = 16                       # max pairs per Hadamard op (2 PSUM banks)
GPS_FRAC = 0.36                # share of elements routed via ACT+GpSimd

# output stages per partition-half as (pair_lo, pair_hi); the two halves run
# on PE row-tiles T0 (partitions 0:64) / T8 (64:128) and are interleaved
# run-by-run so both tiles stream concurrently (~2x PE throughput).
# First stages small to prime the pipe.
_BOUNDS_A = [0, 8, 16, 32, 64, 96, 128, 160, 192, 224, 244, 256]
# B starts at 258 (field 10, low j) so its first multiply doesn't wait for
# the tail of fn0; the two i=9 pairs (256,258) run as a mini-stage at the end
_BOUNDS_B = [258, 264, 272, 288, 320, 352, 384, 416, 448, 472, 488, 496]
STAGES_A = list(zip(_BOUNDS_A[:-1], _BOUNDS_A[1:]))
STAGES_B = list(zip(_BOUNDS_B[:-1], _BOUNDS_B[1:]))

PAIRS = list(combinations(range(F), 2))

_NC_CACHE = {}


def _runs(lo, hi):
    """Runs of consecutive same-group pairs (<=RUN) in [lo,hi), not
    crossing 64-pair W-block boundaries."""
    runs = []
    p = lo
    while p < hi:
        i = PAIRS[p][0]
        e = p
        while (e + 1 < hi and PAIRS[e + 1][0] == i and (e + 1 - p) < RUN
               and (e + 1) % 64 != 0):
            e += 1
        runs.append((p, e - p + 1))
        p = e + 1
    return runs


def _build():
    import concourse.tile as tile
    from concourse import bacc, mybir

    F32 = mybir.dt.float32
    BF16 = mybir.dt.bfloat16
    nc = bacc.Bacc("TRN2", target_bir_lowering=False, debug=False,
                   enable_asserts=False, num_devices=N_CORES)

    wpack = nc.dram_tensor("wpack", [128, 4 * 4096], BF16, kind="ExternalInput").ap()
    featT = nc.dram_tensor("featT", [128, 22 * B_SH], BF16, kind="ExternalInput").ap()
    featN = nc.dram_tensor("featN", [B_SH, F * D], BF16, kind="ExternalInput").ap()
    out = nc.dram_tensor("out", [B_SH, P * D], BF16, kind="ExternalOutput").ap()

    with tile.TileContext(nc) as tc:
        with (
            tc.tile_pool(name="win", bufs=1) as win,
            tc.tile_pool(name="feat", bufs=1) as feat,
            tc.tile_pool(name="stage", bufs=8) as stage_pool,
            tc.tile_pool(name="tmp", bufs=8) as tmp_pool,
            tc.tile_pool(name="psum", bufs=4, space="PSUM") as psum_pool,
        ):
            # resident input tiles ------------------------------------------------
            w = [win.tile([128, 4096], BF16, name=f"w{blk}", tag=f"w{blk}")
                 for blk in range(4)]
            ft = feat.tile([128, 22 * B_SH], BF16, name="ft", tag="ft")
            fn = [feat.tile([128, F * D], BF16, name=f"fn{bc}", tag=f"fn{bc}")
                  for bc in range(2)]

            # issue order = joint demand order of the two interleaved pair
            # streams (A: pairs 0..255 / ft top fields 0..9; B: 256..495 /
            # ft bottom fields 9..30), fine slices first so the first
            # matmuls start ~0.4MB into the input stream. All inputs ride
            # the scalar HWDGE ring; early outputs use the sync ring.
            nc.scalar.dma_start(ft[:, 0:512], featT[:, 0:512])
            nc.scalar.dma_start(w[0][:, 0:1024], wpack[:, 0:1024])
            nc.scalar.dma_start(fn[0][:, 0:1280], featN[0:128, 0:1280])
            nc.scalar.dma_start(w[0][:, 1024:4096], wpack[:, 1024:4096])
            nc.scalar.dma_start(fn[0][:, 1280:2048], featN[0:128, 1280:2048])
            nc.scalar.dma_start(ft[:, 512:1536], featT[:, 512:1536])
            nc.scalar.dma_start(w[1][:, :], wpack[:, 4096:8192])
            nc.scalar.dma_start(ft[:, 1536:2816], featT[:, 1536:2816])
            nc.scalar.dma_start(w[2][:, :], wpack[:, 8192:12288])
            nc.scalar.dma_start(fn[1][:, :], featN[128:256, :])
            # top half of w3 is fully used (pairs 192..255); bottom half only
            # to col 15360 (pair 495) - skip the padding
            nc.scalar.dma_start(w[3][0:64, :], wpack[0:64, 12288:16384])
            nc.scalar.dma_start(w[3][64:128, 0:3072], wpack[64:128, 12288:15360])
            # ft top half (fields 0..9) ends at col 2560; cols 2816+ exist
            # only in the bottom half (fields 20..30) - skip the padding
            nc.scalar.dma_start(ft[64:128, 2816:22 * B_SH],
                                featT[64:128, 2816:22 * B_SH])

            # compute + output ----------------------------------------------------
            state = {"el_tot": 0, "el_gps": 0, "ring": 0}

            def out_dma(dst, src, bc, si):
                # inputs own the scalar ring early in bc=0; after that
                # alternate output DMAs across both HWDGE rings
                if bc == 0 and si < 5:
                    nc.sync.dma_start(dst, src)
                else:
                    eng = nc.sync if state["ring"] % 2 == 0 else nc.scalar
                    state["ring"] += 1
                    eng.dma_start(dst, src)

            def emit_run(p0, n, st, lo, bc, tag):
                i, j0 = PAIRS[p0]
                h = p0 // HALF
                po = 64 * h
                fcol = (i - 9 * h) * B_SH       # field col in ft's half
                colbase = (p0 - h * HALF) * D
                blk, bcol = colbase // 4096, colbase % 4096
                ps = psum_pool.tile([128, RUN * D], F32, tag="ps", bufs=4)
                for k in range(0, n, 8):
                    nk = min(8, n - k)
                    nc.tensor.matmul(
                        ps[:, k * D:(k + nk) * D],
                        lhsT=ft[po:po + 64,
                                fcol + bc * 128: fcol + bc * 128 + 128],
                        rhs=w[blk][po:po + 64,
                                   bcol + k * D: bcol + (k + nk) * D],
                        start=True, stop=True,
                    )
                st_sl = st[:, (p0 - lo) * D: (p0 - lo + n) * D]
                fn_sl = fn[bc][:, j0 * D: (j0 + n) * D]
                # two Hadamard paths (DVE ~104 elem/ns from PSUM; ACT copy
                # + GpSimd ~60 elem/ns), balanced by GPS_FRAC
                state["el_tot"] += n
                if state["el_gps"] < GPS_FRAC * state["el_tot"]:
                    state["el_gps"] += n
                    tmp = tmp_pool.tile([128, RUN * D], BF16, tag="tmp")
                    nc.scalar.copy(tmp[:, 0:n * D], ps[:, 0:n * D])
                    nc.gpsimd.tensor_mul(st_sl, tmp[:, 0:n * D], fn_sl)
                else:
                    nc.vector.tensor_mul(st_sl, ps[:, 0:n * D], fn_sl)

            for bc in range(2):
                for si in range(len(STAGES_A)):
                    aLo, aHi = STAGES_A[si]
                    bLo, bHi = STAGES_B[si]
                    runsA = _runs(aLo, aHi)
                    runsB = _runs(bLo, bHi)
                    stA = stage_pool.tile([128, (aHi - aLo) * D], BF16,
                                          tag="stA", bufs=6)
                    stB = stage_pool.tile([128, (bHi - bLo) * D], BF16,
                                          tag="stB", bufs=6)
                    # alternate halves run-by-run: adjacent matmuls target
                    # different PE row-tiles and execute concurrently
                    for ri in range(max(len(runsA), len(runsB))):
                        if ri < len(runsA):
                            emit_run(*runsA[ri], stA, aLo, bc, "psA")
                        if ri < len(runsB):
                            emit_run(*runsB[ri], stB, bLo, bc, "psB")
                    out_dma(out[bc * 128: bc * 128 + 128, aLo * D: aHi * D],
                            stA[:, :], bc, si)
                    out_dma(out[bc * 128: bc * 128 + 128, bLo * D: bHi * D],
                            stB[:, :], bc, si)
                # deferred i=9 pairs (256,258) of this batch half
                stX = stage_pool.tile([128, 2 * D], BF16, tag="stB", bufs=6)
                emit_run(256, 2, stX, 256, bc, "psB")
                out_dma(out[bc * 128: bc * 128 + 128, 256 * D: 258 * D],
                        stX[:, :], bc, 99)
    nc.compile()
    return nc


def _pack_inputs(feature_emb, W):
    import ml_dtypes

    BF = ml_dtypes.bfloat16
    feature_emb = np.ascontiguousarray(feature_emb, dtype=np.float32)
    W = np.ascontiguousarray(W, dtype=np.float32)
    Wt = W.transpose(0, 2, 1)                      # [P, d, e]
    wpack = np.zeros((128, 4 * 4096), dtype=BF)
    wpack[0:64, :] = Wt[0:HALF].transpose(1, 0, 2).reshape(64, HALF * D).astype(BF)
    wpack[64:128, 0:(P - HALF) * D] = (
        Wt[HALF:P].transpose(1, 0, 2).reshape(64, (P - HALF) * D).astype(BF))
    in_maps = []
    for c in range(N_CORES):
        shard = feature_emb[c * B_SH:(c + 1) * B_SH]         # [256, 32, 64]
        # [d, f, b] per-field transposed features
        ftT = shard.transpose(2, 1, 0).astype(BF)            # [64, 32, 256]
        featT = np.zeros((128, 22 * B_SH), dtype=BF)
        # partitions 0:64 <- fields 0..9 (first-fields of pairs 0..255)
        featT[0:64, 0:10 * B_SH] = ftT[:, 0:10].reshape(64, 10 * B_SH)
        # partitions 64:128 <- fields 9..30 (first-fields of pairs 256..495)
        featT[64:128, :] = ftT[:, 9:31].reshape(64, 22 * B_SH)
        in_maps.append({
            "wpack": wpack,
            "featT": featT,
            "featN": np.ascontiguousarray(shard.reshape(B_SH, F * D).astype(BF)),
        })
    return in_maps


def kernel(feature_emb, W, _trace=False):
    from concourse.bass_utils import run_bass_kernel_spmd

    if "nc" not in _NC_CACHE:
        _NC_CACHE["nc"] = _build()
    nc = _NC_CACHE["nc"]
    in_maps = _pack_inputs(feature_emb, W)
    res = run_bass_kernel_spmd(nc, in_maps, core_ids=list(range(N_CORES)),
                               trace=_trace)
    full = np.concatenate(
        [res.results[c]["out"].astype(np.float32) for c in range(N_CORES)], axis=0)
    out = full.reshape(B, P, D)
    if _trace:
        return out, res
    return out

